# revision 17
# baseline (speedup 1.0000x reference)
"""Trainium2 Bass kernel for BinaryGroupConv block (8-core SPMD, batch-sharded).

For x:(32,256,56,56), w1:(256,64,3,3), w2:(256,256,1,1):
    out = bn1(conv2d(sign(x), sign(w1), s2 p1 g4)) + maxpool3x3s2p1(x)
    x1  = out
    out = bn2(conv2d(sign(out), sign(w2), 1x1)) + x1
with training-mode (batch-stat) BatchNorm -> sync-BN all-reduce across cores.

Strategy per core (4 images, 8 units of [128ch x image]):
  - units ordered t=0 first (0,2,4,6 then 1,3,5,7) so the first half only
    needs w1's first tile; t=1 weight prep is issued mid-loop, hiding the
    late arrival of the second weight tile behind the early units
  - per-unit pipeline: sign (ACT) -> 9-tap block-diag matmuls (PE, stays
    HAM-warm) -> ACT evict + DVE bn_stats on PSUM (lag-2 interleave)
  - maxpool row-stage (DVE, fp16 out) inline; col-stage deferred into the
    sync-BN AllReduce bubble
  - phase2: x1' = a1*y1 + mp (DVE stt); z = sign(x1' + b1) on ACT (+-1)
  - phase4: out = a2*y2 + (b1+b2) + x1' (ACT apply + DVE add), fp16 stores
"""

import contextlib
import sys

import numpy as np

sys.path.insert(0, "/opt/trn_rl_repo")

import concourse.bass as bass
import concourse.tile as tile
from concourse import bacc, mybir
from concourse.bass import ts
from concourse.bass_utils import run_bass_kernel_spmd
from concourse.masks import make_identity

F32 = mybir.dt.float32
BF16 = mybir.dt.bfloat16
FP16 = mybir.dt.float16
AF = mybir.ActivationFunctionType
OP = mybir.AluOpType

EPS = 1e-5
C = 256
H = 56
HO = 28
PIX = HO * HO  # 784
NCHUNK = 392  # matmul/psum N-tile (14 output rows)
RPC = 14  # output rows per chunk
SC1 = 2.0  # conv1: x-sign +/-1 (ACT), w-sign +/-0.5 -> y_true = 2*y_q
SC2 = 2.0  # conv2: z-sign +/-1 (ACT), w-sign +/-0.5 -> y_true = 2*y_q


def build_nc(n_loc: int, n_cores: int):
    nc = bacc.Bacc(
        "TRN2",
        target_bir_lowering=False,
        debug=False,
        enable_asserts=False,
        num_devices=n_cores,
    )
    x_d = nc.dram_tensor("x", [n_loc, C, H, H], F32, kind="ExternalInput").ap()
    w1_d = nc.dram_tensor("w1", [C, 64, 3, 3], F32, kind="ExternalInput").ap()
    w2_d = nc.dram_tensor("w2", [C, C, 1, 1], F32, kind="ExternalInput").ap()
    g1_d = nc.dram_tensor("gamma1", [C], F32, kind="ExternalInput").ap()
    b1_d = nc.dram_tensor("beta1", [C], F32, kind="ExternalInput").ap()
    g2_d = nc.dram_tensor("gamma2", [C], F32, kind="ExternalInput").ap()
    b2_d = nc.dram_tensor("beta2", [C], F32, kind="ExternalInput").ap()
    out_d = nc.dram_tensor("out", [n_loc, C, HO, HO], FP16, kind="ExternalOutput").ap()

    with tile.TileContext(nc) as tc:
        kernel_body(
            tc, out_d, x_d, w1_d, w2_d, (g1_d, b1_d, g2_d, b2_d), n_loc, n_cores
        )

    nc.compile()
    return nc


def kernel_body(tc, out_d, x_d, w1_d, w2_d, gb_d, n_loc, n_cores):
    nc = tc.nc
    g1_d, b1_d, g2_d, b2_d = gb_d
    n_units = n_loc * 2
    npix_loc = n_loc * PIX
    npix_glob = npix_loc * n_cores
    groups = [list(range(n_cores))]
    # t=0 units first: they only need w1 tile 0, which arrives early
    unit_order = [2 * n for n in range(n_loc)] + [2 * n + 1 for n in range(n_loc)]

    ctx = contextlib.ExitStack()
    with ctx:
        singles = ctx.enter_context(tc.tile_pool(name="singles", bufs=1))
        xf_pool = ctx.enter_context(tc.tile_pool(name="xf", bufs=3))
        xs_pool = ctx.enter_context(tc.tile_pool(name="xs", bufs=3))
        rm_pool = ctx.enter_context(tc.tile_pool(name="rm", bufs=n_units))
        mp_pool = ctx.enter_context(tc.tile_pool(name="mp", bufs=n_units))
        y1_pool = ctx.enter_context(tc.tile_pool(name="y1", bufs=n_units))
        z_pool = ctx.enter_context(tc.tile_pool(name="zs", bufs=n_units))
        y2_pool = ctx.enter_context(tc.tile_pool(name="y2", bufs=n_units))
        yst_pool = ctx.enter_context(tc.tile_pool(name="yst", bufs=n_units))
        tiny = ctx.enter_context(tc.tile_pool(name="tiny", bufs=16))
        dram = ctx.enter_context(tc.tile_pool(name="dram", bufs=4, space="DRAM"))
        wprep = ctx.enter_context(tc.tile_pool(name="wprep", bufs=1))

        # Dummy tiny AllReduce issued first: absorbs the cross-core launch
        # skew + collectives-firmware warmup concurrently with phase-1 compute,
        # so the real sync-BN all-reduces are fast.
        warm = tiny.tile([128, 1], F32, tag="warm", name="warm")
        nc.gpsimd.memset(warm, 0.0)
        cc_warm_in = dram.tile([128, 1], F32, tag="ccwi", name="cc_warm_in")
        cc_warm_out = dram.tile([128, 1], F32, tag="ccwo", name="cc_warm_out")
        nc.sync.dma_start(out=cc_warm_in, in_=warm)
        nc.gpsimd.collective_compute(
            "AllReduce",
            OP.add,
            replica_groups=groups,
            ins=[cc_warm_in.opt()],
            outs=[cc_warm_out.opt()],
        )

        # ---- weight DMAs FIRST: they ride ahead of the big x stream ----
        w1nat = [
            wprep.tile([128, 64, 9], F32, tag=f"w1nat{t}", name=f"w1nat{t}")
            for t in range(2)
        ]
        for t in range(2):
            nc.sync.dma_start(
                out=w1nat[t],
                in_=w1_d[ts(t, 128)].rearrange("co ci kh kw -> co ci (kh kw)"),
            )
        w2nat = [
            wprep.tile([128, 256], F32, tag=f"w2nat{m}", name=f"w2nat{m}")
            for m in range(2)
        ]
        for m in range(2):
            nc.sync.dma_start(out=w2nat[m], in_=w2_d[ts(m, 128), :, 0, 0])

        # ---- x tile DMAs in unit order: the ring streams them in order ----
        xf = {}
        for u in unit_order:
            n, t = divmod(u, 2)
            xft = xf_pool.tile([128, H, H], F32, tag="xf", name=f"xf{u}")
            nc.sync.dma_start(out=xft, in_=x_d[n, ts(t, 128)])
            xf[u] = xft

        # ---- gamma/beta stacked [128,2] (col t = channel half) + eps ----
        def load_vec2(d_ap, name):
            tl = singles.tile([128, 2], F32, tag=f"v{name}", name=f"v{name}")
            for t in range(2):
                src = bass.AP(
                    tensor=d_ap.tensor,
                    offset=d_ap.offset + 128 * t,
                    ap=[[1, 128], [0, 1]],
                )
                nc.sync.dma_start(out=tl[:, t : t + 1], in_=src)
            return tl

        g1w = load_vec2(g1_d, "g1")
        b1w = load_vec2(b1_d, "b1")
        g2w = load_vec2(g2_d, "g2")
        b2w = load_vec2(b2_d, "b2")
        eps_t = singles.tile([128, 1], F32, tag="eps", name="eps_t")
        nc.vector.memset(eps_t, EPS)

        # ---- weight prep (split per tile; t=1 issued mid phase-1) ----
        lhsT1 = [
            singles.tile([128, 9, 128], BF16, tag=f"lhsT1_{t}", name=f"lhsT1_{t}")
            for t in range(2)
        ]
        w2lhsT = [
            singles.tile([128, 256], BF16, tag=f"w2lhsT_{k}", name=f"w2lhsT_{k}")
            for k in range(2)
        ]
        ident = singles.tile([128, 128], BF16, tag="ident", name="ident")
        make_identity(nc, ident)
        p1_ctx = contextlib.ExitStack()
        tr_psum = p1_ctx.enter_context(tc.tile_pool(name="trps", bufs=1, space="PSUM"))

        def wprep_w1(t):
            w1ns = wprep.tile([128, 64, 9], BF16, tag=f"w1ns{t}", name=f"w1ns{t}")
            nc.vector.tensor_scalar(
                out=w1ns, in0=w1nat[t], scalar1=0.0, scalar2=0.5,
                op0=OP.is_ge, op1=OP.subtract,
            )
            nc.gpsimd.memset(lhsT1[t], 0.0)
            for tap in range(9):
                trf = tr_psum.tile([128, 128], BF16, tag="trw", name=f"trw{t}{tap}")
                nc.tensor.transpose(trf[0:64], w1ns[:, :, tap], ident)
                nc.tensor.transpose(trf[64:128], w1ns[:, :, tap], ident)
                nc.scalar.copy(out=lhsT1[t][0:64, tap, 0:64], in_=trf[0:64, 0:64])
                nc.vector.tensor_copy(
                    out=lhsT1[t][64:128, tap, 64:128], in_=trf[64:128, 64:128]
                )

        def wprep_w2():
            for mt in range(2):
                w2s = wprep.tile([128, 256], BF16, tag=f"w2s{mt}", name=f"w2s{mt}")
                nc.vector.tensor_scalar(
                    out=w2s, in0=w2nat[mt], scalar1=0.0, scalar2=0.5,
                    op0=OP.is_ge, op1=OP.subtract,
                )
                for kt in range(2):
                    tr = tr_psum.tile([128, 128], BF16, tag="trw", name=f"tr2{mt}{kt}")
                    nc.tensor.transpose(tr, w2s[:, ts(kt, 128)], ident)
                    nc.scalar.copy(out=w2lhsT[kt][:, ts(mt, 128)], in_=tr)

        wprep_w1(0)

        # per-(img,chunk) bn_stats records, one buffer per part-tile
        bnst1 = [
            singles.tile([128, n_units, 6], F32, tag=f"b1_{t}", name=f"bnst1_{t}")
            for t in range(2)
        ]
        bnst2 = [
            singles.tile([128, n_units, 6], F32, tag=f"b2_{t}", name=f"bnst2_{t}")
            for t in range(2)
        ]

        taps = [(kh, kw) for kh in range(3) for kw in range(3)]

        # ------- phase 1: load, sign, pool-s1, conv1, evict+stats -------
        y1_t = {}
        rm_t = {}
        mp_t = {}
        ps_t = {}
        EVICT_LAG = 2

        psum1 = p1_ctx.enter_context(tc.tile_pool(name="psum1", bufs=3, space="PSUM"))
        if True:

            def issue_evict1(u):
                n, t = divmod(u, 2)
                ps = ps_t[u]
                y1 = y1_pool.tile([128, PIX], F32, tag="y1", name=f"y1_{u}")
                for c in range(2):
                    nc.scalar.activation(
                        out=y1[:, ts(c, NCHUNK)],
                        in_=ps[c].rearrange("p a b -> p (a b)"),
                        func=AF.Identity,
                    )
                    nc.vector.bn_stats(
                        out=bnst1[t][:, 2 * n + c, :],
                        in_=ps[c].rearrange("p a b -> p (a b)"),
                    )
                y1_t[u] = y1

            for idx, u in enumerate(unit_order):
                if idx == n_loc:
                    wprep_w1(1)
                    wprep_w2()
                n, t = divmod(u, 2)
                xft = xf[u]

                # binarized input, zero-padded with one row/col at top/left
                xs = xs_pool.tile([128, H + 1, H + 1], BF16, tag="xs", name=f"xs{u}")
                nc.gpsimd.memset(xs[:, 0, :], 0.0)
                nc.gpsimd.memset(xs[:, 1:, 0], 0.0)
                nc.scalar.sign(out=xs[:, 1:, 1:], in_=xft)

                # maxpool row stage (fp16 out: max is a selection, so fp16
                # only rounds the held value; col stage deferred to AR bubble)
                rmax = rm_pool.tile([128, HO, H], FP16, tag="rm", name=f"rm{u}")
                nc.vector.tensor_tensor(
                    out=rmax, in0=xft[:, 0:H:2], in1=xft[:, 1:H:2], op=OP.max
                )
                nc.vector.tensor_tensor(
                    out=rmax[:, 1:], in0=rmax[:, 1:], in1=xft[:, 1 : H - 2 : 2],
                    op=OP.max,
                )
                rm_t[u] = rmax

                # conv1: 9 taps, block-diag [128,128] bf16, PSUM accumulation
                ps = [
                    psum1.tile([128, RPC, HO], F32, tag=f"p1{c}", name=f"ps{u}{c}")
                    for c in range(2)
                ]
                for tap_i, (kh, kw) in enumerate(taps):
                    for c in range(2):
                        rhs = xs[
                            :,
                            28 * c + kh : 28 * c + kh + 27 : 2,
                            kw : kw + 55 : 2,
                        ]
                        nc.tensor.matmul(
                            ps[c],
                            lhsT1[t][:, tap_i, :],
                            rhs,
                            start=(tap_i == 0),
                            stop=(tap_i == 8),
                        )
                ps_t[u] = ps
                if idx >= EVICT_LAG:
                    issue_evict1(unit_order[idx - EVICT_LAG])
            for idx in range(n_units - EVICT_LAG, n_units):
                issue_evict1(unit_order[idx])
        p1_ctx.close()

        # ---- local aggregate -> (S, SS) -> AllReduce ----
        def stats_allreduce(bnst, tag):
            allin = tiny.tile([128, 4], F32, tag=f"ai{tag}", name=f"ai{tag}")
            for t in range(2):
                mv = tiny.tile([128, 2], F32, tag=f"mv{tag}{t}", name=f"mv{tag}{t}")
                nc.vector.bn_aggr(out=mv, in_=bnst[t])
                m2 = tiny.tile([128, 1], F32, tag=f"m2{tag}{t}", name=f"m2{tag}{t}")
                nc.vector.tensor_tensor(
                    out=m2, in0=mv[:, 0:1], in1=mv[:, 0:1], op=OP.mult
                )
                vp = tiny.tile([128, 1], F32, tag=f"vp{tag}{t}", name=f"vp{tag}{t}")
                nc.vector.tensor_tensor(out=vp, in0=mv[:, 1:2], in1=m2, op=OP.add)
                nc.vector.tensor_scalar_mul(
                    out=allin[:, 2 * t : 2 * t + 1], in0=mv[:, 0:1],
                    scalar1=float(npix_loc),
                )
                nc.vector.tensor_scalar_mul(
                    out=allin[:, 2 * t + 1 : 2 * t + 2], in0=vp,
                    scalar1=float(npix_loc),
                )
            cc_in = dram.tile([128, 4], F32, tag=f"ci{tag}", name=f"ci{tag}")
            cc_out = dram.tile([128, 4], F32, tag=f"co{tag}", name=f"co{tag}")
            nc.sync.dma_start(out=cc_in, in_=allin)
            nc.gpsimd.collective_compute(
                "AllReduce",
                OP.add,
                replica_groups=groups,
                ins=[cc_in.opt()],
                outs=[cc_out.opt()],
            )
            gst = tiny.tile([128, 4], F32, tag=f"g{tag}", name=f"g{tag}")
            nc.sync.dma_start(out=gst, in_=cc_out)
            return gst

        def bn_coeffs(gst, gam, bet, SC, tag):
            """Global (S,SS) -> per-tile (a_eff, b_eff) as [128,2] columns."""
            meanq = tiny.tile([128, 2], F32, tag=f"mq{tag}", name=f"mq{tag}")
            nc.vector.tensor_scalar_mul(
                out=meanq, in0=gst[:, 0:4:2], scalar1=1.0 / npix_glob
            )
            ssq = tiny.tile([128, 2], F32, tag=f"sq{tag}", name=f"sq{tag}")
            nc.vector.tensor_scalar_mul(
                out=ssq, in0=gst[:, 1:4:2], scalar1=1.0 / npix_glob
            )
            m2 = tiny.tile([128, 2], F32, tag=f"m2{tag}", name=f"m2{tag}")
            nc.vector.tensor_tensor(out=m2, in0=meanq, in1=meanq, op=OP.mult)
            varq = tiny.tile([128, 2], F32, tag=f"vq{tag}", name=f"vq{tag}")
            nc.vector.tensor_tensor(out=varq, in0=ssq, in1=m2, op=OP.subtract)
            sd = tiny.tile([128, 2], F32, tag=f"sd{tag}", name=f"sd{tag}")
            nc.scalar.activation(
                out=sd, in_=varq, func=AF.Sqrt, bias=eps_t, scale=SC * SC
            )
            r = tiny.tile([128, 2], F32, tag=f"r{tag}", name=f"r{tag}")
            nc.vector.reciprocal(out=r, in_=sd)
            rg = tiny.tile([128, 2], F32, tag=f"rg{tag}", name=f"rg{tag}")
            nc.vector.tensor_tensor(out=rg, in0=r, in1=gam, op=OP.mult)
            a_eff = tiny.tile([128, 2], F32, tag=f"ae{tag}", name=f"ae{tag}")
            nc.vector.tensor_scalar_mul(out=a_eff, in0=rg, scalar1=SC)
            tmp = tiny.tile([128, 2], F32, tag=f"tp{tag}", name=f"tp{tag}")
            nc.vector.tensor_tensor(out=tmp, in0=meanq, in1=rg, op=OP.mult)
            b_eff = tiny.tile([128, 2], F32, tag=f"be{tag}", name=f"be{tag}")
            nc.vector.scalar_tensor_tensor(
                out=b_eff, in0=tmp, scalar=-SC, in1=bet, op0=OP.mult, op1=OP.add
            )
            return a_eff, b_eff

        gst1 = stats_allreduce(bnst1, "s1")

        # deferred maxpool col stage: fills the AR1 bubble (fp16 in -> fp16
        # out is lossless: max is a selection over already-rounded values)
        def issue_pool_s2(u):
            rmax = rm_t[u]
            mp = mp_pool.tile([128, HO, HO], FP16, tag="mp", name=f"mp{u}")
            nc.vector.tensor_tensor(
                out=mp, in0=rmax[:, :, 0:H:2], in1=rmax[:, :, 1:H:2], op=OP.max
            )
            nc.vector.tensor_tensor(
                out=mp[:, :, 1:], in0=mp[:, :, 1:], in1=rmax[:, :, 1 : H - 2 : 2],
                op=OP.max,
            )
            mp_t[u] = mp

        for u in range(n_units):
            issue_pool_s2(u)

        a1, b1 = bn_coeffs(gst1, g1w, b1w, SC1, "c1")

        # ------- phase 2: x1' = a1*y1 + mp; z = sign(x1' + b1) -------
        z_t = {}
        for u in range(n_units):
            n, t = divmod(u, 2)
            y1 = y1_t[u]
            nc.vector.scalar_tensor_tensor(
                out=y1,
                in0=y1,
                scalar=a1[:, t : t + 1],
                in1=mp_t[u].rearrange("p a b -> p (a b)"),
                op0=OP.mult,
                op1=OP.add,
            )
            z = z_pool.tile([128, PIX], BF16, tag="z", name=f"z{u}")
            nc.scalar.sign(out=z, in_=y1, bias=b1[:, t : t + 1])
            z_t[u] = z

        # ------- phase 3: conv2 (1x1), evict + stats -------
        y2_t = {}
        with tc.tile_pool(name="psum2", bufs=4, space="PSUM") as psum2:
            for n in range(n_loc):
                for mt in range(2):
                    ps = [
                        psum2.tile([128, NCHUNK], F32, tag=f"q{c}", name=f"q{n}{mt}{c}")
                        for c in range(2)
                    ]
                    for kt in range(2):
                        for c in range(2):
                            nc.tensor.matmul(
                                ps[c],
                                w2lhsT[kt][:, ts(mt, 128)],
                                z_t[2 * n + kt][:, ts(c, NCHUNK)],
                                start=(kt == 0),
                                stop=(kt == 1),
                            )
                    y2 = y2_pool.tile([128, PIX], F32, tag="y2", name=f"y2_{n}{mt}")
                    for c in range(2):
                        nc.scalar.activation(
                            out=y2[:, ts(c, NCHUNK)], in_=ps[c], func=AF.Identity
                        )
                        nc.vector.bn_stats(
                            out=bnst2[mt][:, 2 * n + c, :], in_=ps[c]
                        )
                    y2_t[(n, mt)] = y2

        gst2 = stats_allreduce(bnst2, "s2")
        a2, b2 = bn_coeffs(gst2, g2w, b2w, SC2, "c2")
        b12 = tiny.tile([128, 2], F32, tag="b12", name="b12")
        nc.vector.tensor_tensor(out=b12, in0=b1, in1=b2, op=OP.add)

        # ------- phase 4: out = a2*y2 + (b1+b2) + x1', fp16 store -------
        for n in range(n_loc):
            for mt in range(2):
                y2 = y2_t[(n, mt)]
                nc.scalar.activation(
                    out=y2, in_=y2, func=AF.Identity,
                    bias=b12[:, mt : mt + 1], scale=a2[:, mt : mt + 1],
                )
                yst = yst_pool.tile([128, PIX], FP16, tag="yst", name=f"yst{n}{mt}")
                nc.vector.tensor_tensor(
                    out=yst, in0=y2, in1=y1_t[2 * n + mt], op=OP.add
                )
                nc.sync.dma_start(
                    out=out_d[n, ts(mt, 128)],
                    in_=yst.rearrange("p (h w) -> p h w", h=HO),
                )


_NC_CACHE = {}


def get_nc(n_loc=4, n_cores=8):
    key = (n_loc, n_cores)
    if key not in _NC_CACHE:
        _NC_CACHE[key] = build_nc(n_loc, n_cores)
    return _NC_CACHE[key]


def kernel(**inputs):
    n_cores = 8
    x = np.asarray(inputs["x"], dtype=np.float32)
    n_loc = x.shape[0] // n_cores
    nc = get_nc(n_loc, n_cores)
    shared = {
        k: np.asarray(inputs[k], dtype=np.float32)
        for k in ("w1", "w2", "gamma1", "beta1", "gamma2", "beta2")
    }
    in_maps = [{"x": x[i * n_loc : (i + 1) * n_loc], **shared} for i in range(n_cores)]
    res = run_bass_kernel_spmd(nc, in_maps, core_ids=list(range(n_cores)))
    return np.concatenate(
        [res.results[i]["out"].astype(np.float32) for i in range(n_cores)], axis=0
    )
